# revision 19
# baseline (speedup 1.0000x reference)
"""Trainium2 Bass kernel for nn_BitwiseModule (scatter_memory) — v3.

The module computes out = x + delta where delta is two one-hot (+1.0)
columns in [80, 112) per active row; all other columns pass through
unchanged.  v1 moved the full 512-col tensor through the device (HBM
roofline ~134 us/core).  v3 moves only what the compute needs:

  host packs   xp = [x[:, 0:4] | x[:, 16:112]]            (100 cols)
  device reads xp, computes slab = x[:, 80:112] + delta   (32 cols)
  host splices out = copy(x); out[:, 80:112] = slab

All arithmetic stays on-device; the host only slices / copies bytes
(the gather/unshard step).  Per-core HBM traffic drops 67 MB -> 8.7 MB.

Packed column map: flags 0:4 (op_and, op_or, op_xor, mark_ax),
fields 4:68 (a_lo, a_hi, b_lo, b_hi; 16 each), base 68:100 (x[:,80:112]).

Per-row pipeline (g rows per partition per supergroup):
  m    = max over v of F[j,f,v]              (DVE tensor_reduce, f32)
  eqh  = (F >= m) -> bf16 {0,1}              (DVE TT, 1x)
  z    = eqh * iotarev(16..1) -> bf16        (DVE TT, 2x fast mode)
  s    = max over v of z  (pairwise TT tree) (DVE TT bf16 2x; first-max
                                              index wins: s = 16 - argmax)
  idx  = 16 - s -> int32                     (tensor_scalar)
  and/or/xor = idx nibble ops                (DVE TT int32, small)
  r    = priority select via copy_predicated; -1 when inactive
  rrep = r repeated 16x                      (ACT copy, so the one-hot
                                              compare gets stride-1 operands)
  d    = onehot16(rrep) -> bf16              (DVE TT is_equal, 2x)
  slab = I.T@base + I.T@d in PSUM            (PE identity matmuls; exact
                                              for f32), ACT copy to SBUF

bf16 holds all intermediate values (0..16 integers) exactly; the fp32
identity matmul through PSUM is bit-exact (verified max abs diff 0.0).
"""

import numpy as np

import bass_rust
import concourse.bass as bass
import concourse.mybir as mybir
from concourse.bass_utils import run_bass_kernel_spmd
from concourse.mybir import AluOpType
from concourse.tile import TileContext
from concourse.vector_clock import ScopedClock

B_FULL = 131072
D = 512
N_CORES = 8
R = B_FULL // N_CORES  # rows per core
P = 128
CP = 100  # packed cols

F32 = mybir.dt.float32
BF16 = mybir.dt.bfloat16
I32 = mybir.dt.int32

CFG = dict(bufs_x=4, bufs_y=3, bufs_m=3, add_engine="pe", split_load=True, schedule=(16, 32, 32, 32, 16))


class SplitDrainTileContext(TileContext):
    """TileContext whose kernel-tail drain spreads its semaphore waits over
    several instructions: the bundled walrus codegen rejects instructions
    carrying more than two sync-wait commands."""

    def _drain_and_barrier(self, tick_clock, wait_clock):
        nc = self.nc
        drain_inst = nc.sync.drain()
        wait_clock.add_sem_waits(
            drain_inst.ins, ScopedClock({None: tick_clock.global_clock})
        )
        si = drain_inst.ins.sync_info
        if si is not None and len(si.on_wait) > 1:
            waits = list(si.on_wait)
            drain_inst.ins.sync_info = bass_rust.SyncInfo(
                on_wait=[waits[0]], on_update=list(si.on_update)
            )
            for w in waits[1:]:
                nop = nc.sync.nop()
                nop.ins.sync_info = bass_rust.SyncInfo(on_wait=[w], on_update=[])
        nc.all_engine_barrier()
        popped = nc._tile_sem_poison_stack.pop()
        assert popped is self._sem_poison
        nc.clear_and_free_semaphores(list(self.sems.allocated().values()))
        nc.all_engine_barrier()


def split_multi_waits(nc: bass.Bass, max_waits: int = 1) -> int:
    """Move surplus sync-waits onto fresh same-engine NoOps inserted before
    the offending instruction (the walrus codegen rejects >1-2 waits)."""
    n_split = 0
    for f in nc.m.functions:
        for blk in f.blocks:
            insts = blk.instructions
            i = 0
            while i < len(insts):
                inst = insts[i]
                si = getattr(inst, "sync_info", None)
                if si is not None and len(si.on_wait) > max_waits:
                    waits = list(si.on_wait)
                    inst.sync_info = bass_rust.SyncInfo(
                        on_wait=waits[:max_waits], on_update=list(si.on_update)
                    )
                    nops = []
                    for k, w in enumerate(waits[max_waits:]):
                        nop = mybir.InstNoOp(
                            name=f"{inst.name}-wsplit{k}",
                            engine=inst.engine,
                            bass_nofuse=True,
                            ins=[],
                            outs=[],
                            sync_info=mybir.SyncInfo(on_wait=[w], on_update=[]),
                        )
                        nc.register_instruction(nop)
                        nops.append(nop)
                    insts[i:i] = nops
                    i += len(nops)
                    n_split += 1
                i += 1
    return n_split


def build_kernel(rows: int = R, g: int = 64, bufs_x: int = 2, bufs_y: int = 2,
                 bufs_m: int = 2, add_engine: str = "v",
                 split_load: bool = True, schedule=None) -> bass.Bass:
    if schedule is None:
        assert rows % (P * g) == 0
        schedule = [g] * (rows // (P * g))
    assert sum(schedule) * P == rows, (schedule, rows)
    gmax = max(schedule)

    nc = bass.Bass(trn_type="TRN2")
    x = nc.dram_tensor("x", [rows, CP], F32, kind="ExternalInput")
    y = nc.dram_tensor("y", [rows, 32], F32, kind="ExternalOutput")

    with SplitDrainTileContext(nc) as tc:
        with (
            tc.tile_pool(name="const", bufs=1) as cpool,
            tc.tile_pool(name="x", bufs=bufs_x) as xpool,
            tc.tile_pool(name="y", bufs=bufs_y) as ypool,
            tc.tile_pool(name="mid", bufs=bufs_m) as mpool,
            tc.tile_pool(name="ps", bufs=4, space=bass.MemorySpace.PSUM) as ppool,
        ):
            # ---- constants ----
            iota_rev_i = cpool.tile([P, 16], I32)  # 16..1
            nc.gpsimd.iota(iota_rev_i[:], pattern=[[-1, 16]], base=16, channel_multiplier=0)
            iota_rev = cpool.tile([P, 16], BF16)
            nc.vector.tensor_copy(iota_rev[:], iota_rev_i[:])
            iota_lh_i = cpool.tile([P, 16], I32)  # 0..15
            nc.gpsimd.iota(iota_lh_i[:], pattern=[[1, 16]], base=0, channel_multiplier=0)
            iota_lh = cpool.tile([P, 16], BF16)
            nc.vector.tensor_copy(iota_lh[:], iota_lh_i[:])
            neg1 = cpool.tile([P, 2 * gmax], I32)
            nc.vector.memset(neg1[:], -1)
            if add_engine == "pe":
                # bf16 identity for PE identity-matmul adds
                col_i = cpool.tile([P, P], I32)
                nc.gpsimd.iota(col_i[:], pattern=[[1, P]], base=0, channel_multiplier=0)
                part_i = cpool.tile([P, P], I32)
                nc.gpsimd.iota(part_i[:], pattern=[[0, P]], base=0, channel_multiplier=1)
                ident = cpool.tile([P, P], BF16)
                nc.vector.tensor_tensor(ident[:], col_i[:], part_i[:], AluOpType.is_equal)
                ident_f = cpool.tile([P, P], F32)
                nc.vector.tensor_tensor(ident_f[:], col_i[:], part_i[:], AluOpType.is_equal)

            start = 0
            for sg, g in enumerate(schedule):
                jh = g // 2
                neg1_3 = neg1[:, 0 : 2 * g].rearrange("p (j h) -> p j h", j=g)
                iota_rev_b = iota_rev[:].unsqueeze(1).broadcast_to((P, g * 4, 16))
                iota_lh_b = iota_lh[:].unsqueeze(1).broadcast_to((P, g * 2, 16))
                x_sg = x[start : start + P * g].rearrange("(p j) d -> p j d", p=P)
                y_sg = y[start : start + P * g].rearrange("(p j) d -> p j d", p=P)
                start += P * g

                X = xpool.tile([P, g * CP], F32, name=f"X{g}")
                X3 = X[:].rearrange("p (j d) -> p j d", j=g)
                if split_load:
                    # split the load across both HWDGE queues
                    nc.sync.dma_start(X3[:, 0:jh, :], x_sg[:, 0:jh, :])
                    nc.scalar.dma_start(X3[:, jh:g, :], x_sg[:, jh:g, :])
                else:
                    leng = nc.scalar if sg % 2 else nc.sync
                    leng.dma_start(X3, x_sg)

                # fields as [p, j, f, v]
                F4 = X3[:, :, 4:68].rearrange("p j (f v) -> p j f v", v=16)

                # per-field max (DVE, f32)
                m = mpool.tile([P, g * 4], F32, name=f"m{g}")
                m3 = m[:].rearrange("p (j f) -> p j f", j=g)
                nc.vector.tensor_reduce(
                    m3, F4, axis=mybir.AxisListType.X, op=AluOpType.max
                )

                # eqh = (F >= m) -> bf16
                eqh = mpool.tile([P, g * 64], BF16, name=f"eqh{g}")
                eqh4 = eqh[:].rearrange("p (j f v) -> p j f v", j=g, f=4)
                eqh3 = eqh[:].rearrange("p (k v) -> p k v", v=16)
                m_b4 = m3.unsqueeze(3).broadcast_to((P, g, 4, 16))
                nc.vector.tensor_tensor(eqh4, F4, m_b4, AluOpType.is_ge)

                # z = eqh * iotarev  (bf16 2x)
                z = mpool.tile([P, g * 64], BF16, name=f"z{g}")
                z3 = z[:].rearrange("p (k v) -> p k v", v=16)
                nc.vector.tensor_tensor(z3, eqh3, iota_rev_b, AluOpType.mult)

                # s = max over v via pairwise TT tree (bf16 2x)
                t1 = mpool.tile([P, g * 32], BF16, name=f"t1{g}")
                t13 = t1[:].rearrange("p (k v) -> p k v", v=8)
                nc.vector.tensor_tensor(t13, z3[:, :, 0:8], z3[:, :, 8:16], AluOpType.max)
                t2 = mpool.tile([P, g * 16], BF16, name=f"t2{g}")
                t23 = t2[:].rearrange("p (k v) -> p k v", v=4)
                nc.vector.tensor_tensor(t23, t13[:, :, 0:4], t13[:, :, 4:8], AluOpType.max)
                t3 = mpool.tile([P, g * 8], BF16, name=f"t3{g}")
                t33 = t3[:].rearrange("p (k v) -> p k v", v=2)
                nc.vector.tensor_tensor(t33, t23[:, :, 0:2], t23[:, :, 2:4], AluOpType.max)
                s = mpool.tile([P, g * 4], BF16, name=f"s{g}")
                s3 = s[:].rearrange("p (k v) -> p k v", v=1)
                nc.vector.tensor_tensor(s3, t33[:, :, 0:1], t33[:, :, 1:2], AluOpType.max)

                # idx = 16 - s  (int32)
                idx = mpool.tile([P, g * 4], I32, name=f"idx{g}")
                idx3 = idx[:].rearrange("p (j f) -> p j f", j=g)
                nc.vector.tensor_scalar(idx[:], s[:], -1.0, 16.0, AluOpType.mult, AluOpType.add)

                # nibble-wise bitwise ops (int32, small)
                a2 = idx3[:, :, 0:2]
                b2 = idx3[:, :, 2:4]
                and_t = mpool.tile([P, g * 2], I32, name=f"and_t{g}")
                and3 = and_t[:].rearrange("p (j h) -> p j h", j=g)
                nc.vector.tensor_tensor(and3, a2, b2, AluOpType.bitwise_and)
                or_t = mpool.tile([P, g * 2], I32, name=f"or_t{g}")
                or3 = or_t[:].rearrange("p (j h) -> p j h", j=g)
                nc.vector.tensor_tensor(or3, a2, b2, AluOpType.bitwise_or)
                xor_t = mpool.tile([P, g * 2], I32, name=f"xor_t{g}")
                xor3 = xor_t[:].rearrange("p (j h) -> p j h", j=g)
                nc.vector.tensor_tensor(xor3, a2, b2, AluOpType.bitwise_xor)

                # active flags duplicated per (lo, hi); the three is_gt flags
                # (op_and, op_or, op_xor) batch into one op
                gfl = mpool.tile([P, g * 6], I32, name=f"gfl{g}")
                gfl4 = gfl[:].rearrange("p (j c h) -> p j c h", j=g, c=3)
                src3 = X3[:, :, 0:3].unsqueeze(3).broadcast_to((P, g, 3, 2))
                nc.vector.tensor_scalar(gfl4, src3, 0.5, None, AluOpType.is_gt)
                ga = gfl4[:, :, 0, :]
                go = gfl4[:, :, 1, :]
                gx = gfl4[:, :, 2, :]
                gm_t = mpool.tile([P, g * 2], I32, name=f"gm_t{g}")
                gm_n = gm_t[:].rearrange("p (j h) -> p j h", j=g)
                nc.vector.tensor_scalar(
                    gm_n, X3[:, :, 3:4].broadcast_to((P, g, 2)), 0.5, None,
                    AluOpType.is_le,
                )

                # priority select xor > or > and; -1 when inactive
                r = mpool.tile([P, g * 2], I32, name=f"r{g}")
                r3 = r[:].rearrange("p (j h) -> p j h", j=g)
                nc.vector.memset(r[:], -1)
                nc.vector.copy_predicated(r3, ga, and3)
                nc.vector.copy_predicated(r3, go, or3)
                nc.vector.copy_predicated(r3, gx, xor3)
                nc.vector.copy_predicated(r3, gm_n, neg1_3)

                # materialize r repeated 16x on the idle ACT engine so the
                # one-hot compare has stride-1 operands (DVE 2x fast mode)
                rrep = mpool.tile([P, g * 32], BF16, name=f"rrep{g}")
                rrep3 = rrep[:].rearrange("p (k v) -> p k v", v=16)
                r_bc = r[:].unsqueeze(2).broadcast_to((P, g * 2, 16))
                nc.scalar.copy(rrep3, r_bc)

                # one-hot delta (bf16)
                d = mpool.tile([P, g * 32], BF16, name=f"d{g}")
                d3h = d[:].rearrange("p (k v) -> p k v", v=16)
                nc.vector.tensor_tensor(d3h, iota_lh_b, rrep3, AluOpType.is_equal)

                # slab = base + delta  -> f32
                d3 = d[:].rearrange("p (j w) -> p j w", j=g)
                if add_engine == "pe":
                    # psum = I.T @ base + I.T @ delta on the idle PE engine,
                    # one PSUM bank (512 f32) per half-supergroup
                    for h, eng in ((0, nc.sync), (1, nc.scalar)):
                        j0, j1 = h * jh, (h + 1) * jh
                        pt = ppool.tile([P, jh * 32], F32, name=f"pt{g}")
                        pt3 = pt[:].rearrange("p (j w) -> p j w", j=jh)
                        nc.tensor.matmul(
                            pt3, ident_f[:], X3[:, j0:j1, 68:100],
                            start=True, stop=False,
                        )
                        nc.tensor.matmul(
                            pt3, ident[:], d3[:, j0:j1, :],
                            start=False, stop=True,
                        )
                        Yh = ypool.tile([P, jh * 32], F32, name=f"Yh{g}")
                        nc.scalar.copy(Yh[:], pt[:])
                        eng.dma_start(
                            y_sg[:, j0:j1, :],
                            Yh[:].rearrange("p (j w) -> p j w", j=jh),
                        )
                else:
                    Y = ypool.tile([P, g * 32], F32, name=f"Y{g}")
                    Y3 = Y[:].rearrange("p (j w) -> p j w", j=g)
                    add_eng = nc.vector if add_engine == "v" else nc.gpsimd
                    add_eng.tensor_tensor(Y3, X3[:, :, 68:100], d3, AluOpType.add)

                    # split the store across both HWDGE queues
                    nc.sync.dma_start(y_sg[:, 0:jh, :], Y3[:, 0:jh, :])
                    nc.scalar.dma_start(y_sg[:, jh:g, :], Y3[:, jh:g, :])

    split_multi_waits(nc)
    return nc


_CACHED = {}


def _get_kernel():
    key = tuple(sorted((k, tuple(v) if isinstance(v, (tuple, list)) else v) for k, v in CFG.items()))
    if key not in _CACHED:
        cfg = dict(CFG)
        if "schedule" in cfg:
            cfg["schedule"] = list(cfg["schedule"])
        _CACHED[key] = build_kernel(R, **cfg)
    return _CACHED[key]


def kernel(x: np.ndarray, _trace: bool = False):
    x = np.asarray(x)
    assert x.shape == (B_FULL, D), x.shape
    nc = _get_kernel()
    xp = np.empty((B_FULL, CP), dtype=np.float32)
    xp[:, 0:4] = x[:, 0:4]
    xp[:, 4:CP] = x[:, 16:112]
    in_maps = [{"x": xp[i * R : (i + 1) * R]} for i in range(N_CORES)]
    res = run_bass_kernel_spmd(
        nc, in_maps, core_ids=list(range(N_CORES)), trace=_trace
    )
    out = np.array(x, dtype=np.float32, copy=True)
    out[:, 80:112] = np.concatenate(
        [res.results[i]["y"] for i in range(N_CORES)], axis=0
    )
    if _trace:
        kernel._last_results = res
    return out
